# revision 16
# baseline (speedup 1.0000x reference)
"""DecoderLayer (spatial attn + causal temporal attn + FFN) on 8 TRN2 NeuronCores.

Sharding: data-parallel. Spatial attention shards T (16 t-slices/core);
an on-device AllToAll reshards to N (32 cells/core) for causal temporal
attention + FFN. Activations are kept feature-major ([d, token]) on chip so
every matmul consumes natural layouts; the host pre/post-transposes x/out.

Matmul dtypes: fp32r (full-rate fp32 at free-dim>=256) for projections/FFN
and spatial scores; bf16 for softmax weights, V, and temporal scores.
Attention processes head PAIRS so softmax/normalize elementwise ops run at
the full 128-partition width.
"""
import sys

sys.path.insert(0, "/opt/trn_rl_repo")
import numpy as np
import concourse.bass as bass
import concourse.mybir as mybir
import concourse.tile as tile
from concourse import bacc
from concourse import bass_utils
from concourse.masks import make_upper_triangular

F32 = mybir.dt.float32
F32R = mybir.dt.float32r
BF16 = mybir.dt.bfloat16
EXP = mybir.ActivationFunctionType.Exp
SQUARE = mybir.ActivationFunctionType.Square
SQRT = mybir.ActivationFunctionType.Sqrt
IDENT = mybir.ActivationFunctionType.Identity
RELU = mybir.ActivationFunctionType.Relu
ADD = mybir.AluOpType.add
MULT = mybir.AluOpType.mult

T, N, D, H, DK, DFF = 128, 256, 512, 8, 64, 2048
NC = 8
TS = T // NC          # 16 t-slices per core (spatial)
NS = N // NC          # 32 cells per core (temporal)
TOK = TS * N          # 4096 tokens per core
NB = 8                # temporal batch: cells per batch
NBATCH = NS // NB     # 4 temporal batches
BTOK = T * NB         # 1024 tokens per temporal batch


def _mm(nc, out, lhsT, rhs, start, stop):
    nc.tensor.matmul(out, lhsT, rhs, start=start, stop=stop)


def _load_w(nc, sb, dram_ap, kdim, ndim, tag, dt=F32R):
    """Weight [kdim, ndim] -> SBUF [128, (kdim//128)*ndim], chunk dc at
    cols [dc*ndim, (dc+1)*ndim). Single multi-dim DMA."""
    kc = kdim // 128
    t = sb.tile([128, kc * ndim], dt, tag=tag, name=tag)
    nc.sync.dma_start(
        t[:], dram_ap.rearrange("(dc p) n -> p dc n", p=128).bitcast(dt))
    return t


def _attn_weights(nc, wp, w, p):
    wq = _load_w(nc, wp, w[p + "qw"], D, D, "wq")
    wk = _load_w(nc, wp, w[p + "kw"], D, D, "wk")
    wv = _load_w(nc, wp, w[p + "vw"], D, D, "wv")
    wo = _load_w(nc, wp, w[p + "ow"], D, D, "wo")
    bias = {}
    for m in ("q", "k", "o"):
        bt = wp.tile([128, 4], F32, tag="b" + m, name="b" + m)
        nc.sync.dma_start(bt[:], w[p + m + "b"])
        bias[m] = bt
    bvb = wp.tile([128, D], F32, tag="bvb", name="bvb")
    nc.sync.dma_start(bvb[:], w[p + "vbb"])
    return wq, wk, wv, wo, bias, bvb


def build():
    nc = bacc.Bacc("TRN2", target_bir_lowering=False, debug=False,
                   num_devices=NC)

    def inp(name, shape):
        return nc.dram_tensor(name, shape, F32, kind="ExternalInput").ap()

    xs = inp("xs", [D, TOK])                       # spatial shard, feature-major
    w = {}
    for p in ("s", "t"):
        for m in ("q", "k", "v", "o"):
            w[p + m + "w"] = inp(p + m + "w", [D, D])
        for m in ("q", "k", "o"):
            w[p + m + "b"] = inp(p + m + "b", [128, 4])
        w[p + "vbb"] = inp(p + "vbb", [128, D])
    f1w = inp("f1w", [D, DFF])
    f1b = inp("f1b", [128, DFF // 128])
    f2w = inp("f2w", [DFF, D])
    f2b = inp("f2b", [128, 4])
    lngb = inp("lngb", [2, D])                     # row0 = ln3_g, row1 = ln3_b
    ones512 = inp("ones512", [1, 512])
    ones128 = inp("ones128", [128, 1])
    out = nc.dram_tensor("out", [D, TOK], F32, kind="ExternalOutput").ap()

    with tile.TileContext(nc) as tc, \
         nc.allow_low_precision(reason="bf16 softmax weights and values"):
        with tc.tile_pool(name="const", bufs=1) as cb, \
             tc.tile_pool(name="dram", bufs=1, space="DRAM") as dram:
            ones_bf = cb.tile([128, 1], BF16, tag="obf")
            nc.vector.memset(ones_bf[:], 1.0)
            ones_r1 = cb.tile([1, 512], F32R, tag="o1r")
            nc.sync.dma_start(ones_r1[:], ones512.bitcast(F32R))
            ones_r128 = cb.tile([128, 1], F32R, tag="o128r")
            nc.sync.dma_start(ones_r128[:], ones128.bitcast(F32R))
            mask = cb.tile([128, 128], BF16, tag="mask")
            make_upper_triangular(nc, mask[:], val=1.0, diag=True)
            mask4 = cb.tile([128, 512], BF16, tag="mask4")
            for i in range(4):
                nc.vector.tensor_copy(mask4[:, i * 128:(i + 1) * 128], mask[:])
            lgb = cb.tile([2, D], F32R, tag="lgb")
            nc.sync.dma_start(lgb[:], lngb.bitcast(F32R))

            # a2a buffers: [dst, tloc(4), dc, p, n(32)] per quarter
            q_shape = [NC, TS // 4, 4, 128, 32]
            sends = [dram.tile(q_shape, F32, name=f"send{i}",
                               tag=f"send{i}") for i in range(4)]
            recvs = [dram.tile(q_shape, F32, name=f"recv{i}",
                               tag=f"recv{i}") for i in range(4)]
            r2d = dram.tile([4, 128, TOK], F32, name="r2d", tag="r2d")

            # ---------------- Phase A: spatial attention over n (shard T) ---
            with tc.tile_pool(name="wA", bufs=1) as wp, \
                 tc.tile_pool(name="actA", bufs=2) as ap, \
                 tc.tile_pool(name="smA", bufs=4) as sc, \
                 tc.tile_pool(name="psA", bufs=2, space="PSUM") as pp:
                wq, wk, wv, wo, bias, bvb = _attn_weights(nc, wp, w, "s")

                for tp in range(TS // 2):   # 8 pairs of t-slices, 512 tok each
                    xt = ap.tile([128, 2048], F32R, tag="xt")
                    nc.sync.dma_start(
                        xt[:], xs.rearrange("(dc p) f -> p dc f", p=128)
                        [:, :, tp * 512:(tp + 1) * 512].bitcast(F32R))
                    qt = ap.tile([128, 2048], F32R, tag="qt")
                    kt = ap.tile([128, 2048], F32R, tag="kt")
                    for mo in range(4):
                        qps = pp.tile([128, 512], F32, tag="pa")
                        kps = pp.tile([128, 512], F32, tag="pb")
                        for dc in range(4):
                            _mm(nc, qps[:],
                                wq[:, dc * D + mo * 128: dc * D + (mo + 1) * 128],
                                xt[:, dc * 512:(dc + 1) * 512], dc == 0, dc == 3)
                            _mm(nc, kps[:],
                                wk[:, dc * D + mo * 128: dc * D + (mo + 1) * 128],
                                xt[:, dc * 512:(dc + 1) * 512], dc == 0, dc == 3)
                        nc.scalar.activation(qt[:, mo * 512:(mo + 1) * 512],
                                             qps[:], IDENT,
                                             bias=bias["q"][:, mo:mo + 1])
                        nc.vector.tensor_scalar_add(
                            kt[:, mo * 512:(mo + 1) * 512], kps[:],
                            bias["k"][:, mo:mo + 1])
                    # V token-major, bf16: chunk c (128 tokens) at cols c*512..
                    vt = ap.tile([128, 2048], BF16, tag="vt")
                    for c in range(4):
                        vps = pp.tile([128, 512], F32, tag="pa")
                        for dc in range(4):
                            _mm(nc, vps[:],
                                xt[:, dc * 512 + c * 128: dc * 512 + (c + 1) * 128],
                                wv[:, dc * D:(dc + 1) * D], dc == 0, dc == 3)
                        nc.vector.tensor_add(vt[:, c * 512:(c + 1) * 512],
                                             vps[:], bvb[:])
                    ot = ap.tile([128, 2048], F32R, tag="ot")
                    for hp in range(4):           # head pair chunk
                        for hh in range(2):
                            h = 2 * hp + hh
                            po = 64 * hh
                            es_d = {}
                            dps = pp.tile([1, 512], F32, tag="pd")
                            for tl in range(2):
                                base = hp * 512 + tl * 256
                                sps = pp.tile([128, 512], F32, tag="pb")
                                for kc in range(2):
                                    _mm(nc, sps[:, kc * 256:(kc + 1) * 256],
                                        kt[po:po + 64,
                                           base + kc * 128: base + (kc + 1) * 128],
                                        qt[po:po + 64, base:base + 256],
                                        True, True)
                                es = sc.tile([128, 512], BF16, tag="es")
                                nc.scalar.activation(es[:], sps[:], EXP)
                                for kc in range(2):
                                    _mm(nc, dps[:, tl * 256:(tl + 1) * 256],
                                        ones_bf[:],
                                        es[:, kc * 256:(kc + 1) * 256],
                                        kc == 0, kc == 1)
                                es_d[tl] = es
                            recf = sc.tile([1, 512], F32, tag="recf")
                            nc.vector.reciprocal_approx_fast(recf[:], dps[:])
                            rec = sc.tile([1, 512], F32R, tag="rec")
                            nc.scalar.copy(rec[:], recf[:])
                            bps = pp.tile([64, 512], F32, tag="pe")
                            _mm(nc, bps[:], ones_r1[:, :64], rec[:], True, True)
                            ops = pp.tile([64, 512], F32, tag="pa")
                            for tl in range(2):
                                for kc in range(2):
                                    _mm(nc, ops[:, tl * 256:(tl + 1) * 256],
                                        vt[:, (2 * tl + kc) * 512 + h * 64:
                                           (2 * tl + kc) * 512 + (h + 1) * 64],
                                        es_d[tl][:, kc * 256:(kc + 1) * 256],
                                        kc == 0, kc == 1)
                            osl = ot[po:po + 64, hp * 512:(hp + 1) * 512]
                            nc.scalar.copy(osl, ops[:])
                            nc.vector.tensor_mul(osl, osl, bps[:])
                    r1 = ap.tile([128, 2048], F32, tag="r1")
                    for mo in range(4):
                        rps = pp.tile([128, 512], F32, tag="pb")
                        for dc in range(4):
                            _mm(nc, rps[:],
                                wo[:, dc * D + mo * 128: dc * D + (mo + 1) * 128],
                                ot[:, dc * 512:(dc + 1) * 512], dc == 0, dc == 3)
                        nc.vector.scalar_tensor_tensor(
                            r1[:, mo * 512:(mo + 1) * 512], rps[:],
                            bias["o"][:, mo:mo + 1],
                            xt[:, mo * 512:(mo + 1) * 512].bitcast(F32),
                            ADD, ADD)
                    # scatter to a2a send buffers: 1 DMA per t-slice
                    r1v = r1.rearrange("p (dc tl j n) -> p tl dc j n",
                                       dc=4, tl=2, j=8)
                    for tl in range(2):
                        t = 2 * tp + tl
                        sh, tloc = t // 4, t % 4
                        for j in range(NC):
                            nc.sync.dma_start(
                                sends[sh][j, tloc].transpose([1, 0, 2]),
                                r1v[:, tl, :, j])
                    if tp % 2 == 1:
                        q = tp // 2
                        nc.gpsimd.collective_compute(
                            "AllToAll", mybir.AluOpType.bypass,
                            replica_groups=[list(range(NC))],
                            ins=[sends[q][:].opt()], outs=[recvs[q][:].opt()])

            # ---------------- Phase B: temporal attention over t (shard N) --
            with tc.tile_pool(name="wB", bufs=1) as wp, \
                 tc.tile_pool(name="actBx", bufs=2) as apx, \
                 tc.tile_pool(name="actB", bufs=1) as ap, \
                 tc.tile_pool(name="smB", bufs=4) as sc, \
                 tc.tile_pool(name="psB", bufs=2, space="PSUM") as pp:
                wq, wk, wv, wo, bias, bvb = _attn_weights(nc, wp, w, "t")

                for b in range(NBATCH):   # 8 cells per batch, 1024 tokens
                    xt = apx.tile([128, 4 * BTOK], F32R, tag="xt")
                    xtv = xt.rearrange("p (dc s2 q4 tl nl) -> p dc s2 q4 tl nl",
                                       dc=4, s2=8, q4=4, nl=NB)
                    for src in range(NC):
                        for qr in range(4):
                            for dc in range(4):
                                nc.sync.dma_start(
                                    xtv[:, dc, src, qr],
                                    recvs[qr][src, :, dc, :,
                                              b * NB:(b + 1) * NB]
                                    .transpose([1, 0, 2]).bitcast(F32R))
                    qt = ap.tile([128, 4 * BTOK], BF16, tag="qt")
                    kt = ap.tile([128, 4 * BTOK], BF16, tag="kt")
                    for mo in range(4):
                        for sl in range(2):
                            qps = pp.tile([128, 512], F32, tag="pa")
                            kps = pp.tile([128, 512], F32, tag="pb")
                            for dc in range(4):
                                _mm(nc, qps[:],
                                    wq[:, dc * D + mo * 128: dc * D + (mo + 1) * 128],
                                    xt[:, dc * BTOK + sl * 512:
                                       dc * BTOK + (sl + 1) * 512], dc == 0, dc == 3)
                                _mm(nc, kps[:],
                                    wk[:, dc * D + mo * 128: dc * D + (mo + 1) * 128],
                                    xt[:, dc * BTOK + sl * 512:
                                       dc * BTOK + (sl + 1) * 512], dc == 0, dc == 3)
                            nc.scalar.activation(
                                qt[:, mo * BTOK + sl * 512: mo * BTOK + (sl + 1) * 512],
                                qps[:], IDENT, bias=bias["q"][:, mo:mo + 1])
                            nc.vector.tensor_scalar_add(
                                kt[:, mo * BTOK + sl * 512: mo * BTOK + (sl + 1) * 512],
                                kps[:], bias["k"][:, mo:mo + 1])
                    vt = ap.tile([128, NB * 512], BF16, tag="vt")
                    for nl in range(NB):
                        vps = pp.tile([128, 512], F32, tag="pa")
                        for dc in range(4):
                            lhsT = xt[:, dc * BTOK:(dc + 1) * BTOK].rearrange(
                                "p (t n) -> p t n", n=NB)[:, :, nl]
                            _mm(nc, vps[:], lhsT, wv[:, dc * D:(dc + 1) * D],
                                dc == 0, dc == 3)
                        nc.vector.tensor_add(vt[:, nl * 512:(nl + 1) * 512],
                                             vps[:], bvb[:])
                    ot = ap.tile([128, 4 * BTOK], F32R, tag="ot")
                    for quad in range(NB // 4):
                        n1 = 4 * quad
                        for h in range(H):
                            po, dch = (h % 2) * 64, h // 2
                            qh = qt[:, dch * BTOK:(dch + 1) * BTOK].rearrange(
                                "p (t n) -> p t n", n=NB)
                            kh = kt[:, dch * BTOK:(dch + 1) * BTOK].rearrange(
                                "p (t n) -> p t n", n=NB)
                            # sps blocks: 4 cells of head h
                            sps = pp.tile([128, 512], F32, tag="pb")
                            for j in range(4):
                                _mm(nc, sps[:, j * 128:(j + 1) * 128],
                                    kh[po:po + 64, :, n1 + j],
                                    qh[po:po + 64, :, n1 + j], True, True)
                            es = sc.tile([128, 512], BF16, tag="es")
                            nc.scalar.activation(es[:], sps[:], EXP)
                            esm = sc.tile([128, 512], BF16, tag="esm")
                            nc.vector.tensor_mul(esm[:], es[:], mask4[:])
                            dps = pp.tile([1, 512], F32, tag="pd")
                            _mm(nc, dps[:], ones_bf[:], esm[:], True, True)
                            recf = sc.tile([1, 512], F32, tag="recf")
                            nc.vector.reciprocal_approx_fast(recf[:], dps[:])
                            rec = sc.tile([1, 512], F32R, tag="rec")
                            nc.scalar.copy(rec[:], recf[:])
                            bps = pp.tile([64, 512], F32, tag="pe")
                            _mm(nc, bps[:], ones_r1[:, :64], rec[:], True, True)
                            ops = pp.tile([64, 512], F32, tag="pa")
                            for j in range(4):
                                _mm(nc, ops[:, j * 128:(j + 1) * 128],
                                    vt[:, (n1 + j) * 512 + h * 64:
                                       (n1 + j) * 512 + (h + 1) * 64],
                                    esm[:, j * 128:(j + 1) * 128], True, True)
                            osl = ot[:, dch * BTOK:(dch + 1) * BTOK].rearrange(
                                "p (t n) -> p n t", n=NB)[po:po + 64,
                                                          n1:n1 + 4, :]
                            nc.scalar.copy(osl, ops[:])
                            nc.vector.tensor_mul(osl, osl, bps[:])
                    r2 = ap.tile([128, 4 * BTOK], F32, tag="r2")
                    for mo in range(4):
                        for sl in range(2):
                            rps = pp.tile([128, 512], F32, tag="pb")
                            for dc in range(4):
                                _mm(nc, rps[:],
                                    wo[:, dc * D + mo * 128: dc * D + (mo + 1) * 128],
                                    ot[:, dc * BTOK + sl * 512:
                                       dc * BTOK + (sl + 1) * 512], dc == 0, dc == 3)
                            nc.vector.scalar_tensor_tensor(
                                r2[:, mo * BTOK + sl * 512: mo * BTOK + (sl + 1) * 512],
                                rps[:], bias["o"][:, mo:mo + 1],
                                xt[:, mo * BTOK + sl * 512:
                                   mo * BTOK + (sl + 1) * 512].bitcast(F32),
                                ADD, ADD)
                    r2v = r2.rearrange("p (dc t n) -> p dc n t", dc=4, n=NB)
                    for dc in range(4):
                        for nl in range(NB):
                            f0 = (b * NB + nl) * T
                            nc.sync.dma_start(r2d[dc][:, f0:f0 + T],
                                              r2v[:, dc, nl])

            # ---------------- Phase B2: LN + FFN (token-parallel) -----------
            with tc.tile_pool(name="wC", bufs=1) as wp, \
                 tc.tile_pool(name="actCx", bufs=2) as apx, \
                 tc.tile_pool(name="actC", bufs=1) as ap, \
                 tc.tile_pool(name="smC", bufs=3) as sc, \
                 tc.tile_pool(name="smC1", bufs=1) as sc1, \
                 tc.tile_pool(name="psC", bufs=2, space="PSUM") as pp, \
                 tc.tile_pool(name="psC1", bufs=1, space="PSUM") as pp1:
                f1t = _load_w(nc, wp, f1w, D, DFF, "f1")
                f2t = _load_w(nc, wp, f2w, DFF, D, "f2")
                b1 = wp.tile([128, 16], F32, tag="b1")
                nc.sync.dma_start(b1[:], f1b)
                b2 = wp.tile([128, 4], F32, tag="b2")
                nc.sync.dma_start(b2[:], f2b)

                for s in range(TOK // 512):
                    xt = apx.tile([128, 2048], F32R, tag="xt")
                    nc.sync.dma_start(
                        xt[:], r2d[:, :, s * 512:(s + 1) * 512]
                        .transpose([1, 0, 2]).bitcast(F32R))
                    sps = pp1.tile([1, 512], F32, tag="st0")
                    sqps = pp1.tile([1, 512], F32, tag="st1")
                    for dc in range(4):
                        sq = sc.tile([128, 512], F32R, tag="sq")
                        nc.scalar.activation(
                            sq[:], xt[:, dc * 512:(dc + 1) * 512].bitcast(F32),
                            SQUARE)
                        _mm(nc, sps[:], ones_r128[:],
                            xt[:, dc * 512:(dc + 1) * 512], dc == 0, dc == 3)
                        _mm(nc, sqps[:], ones_r128[:], sq[:], dc == 0, dc == 3)
                    nmean = sc1.tile([1, 512], F32, tag="nm")
                    nc.vector.tensor_scalar_mul(nmean[:], sps[:], -1.0 / D)
                    msq = sc1.tile([1, 512], F32, tag="mq")
                    nc.vector.tensor_scalar_mul(msq[:], sqps[:], 1.0 / D)
                    m2 = sc1.tile([1, 512], F32, tag="m2")
                    nc.vector.tensor_mul(m2[:], nmean[:], nmean[:])
                    var = sc1.tile([1, 512], F32, tag="va")
                    nc.vector.tensor_sub(var[:], msq[:], m2[:])
                    nc.vector.tensor_scalar_add(var[:], var[:], 1e-5)
                    sd = sc1.tile([1, 512], F32, tag="sd")
                    nc.scalar.activation(sd[:], var[:], SQRT)
                    rstdf = sc1.tile([1, 512], F32, tag="rsf")
                    nc.vector.reciprocal_approx_fast(rstdf[:], sd[:])
                    rstd = sc1.tile([1, 512], F32R, tag="rs")
                    nc.scalar.copy(rstd[:], rstdf[:])
                    cr = sc1.tile([2, 512], F32R, tag="cr")
                    nc.vector.tensor_mul(cr[0:1, :], nmean[:], rstdf[:])
                    nc.sync.dma_start(cr[1:2, :], ones512.bitcast(F32R))
                    # F = g (x) rstd ; E = g (x) cneg + b (x) 1   (broadcasts)
                    yt = ap.tile([128, 2048], F32R, tag="yt")
                    for dc in range(4):
                        fps = pp1.tile([128, 512], F32, tag="bc0")
                        _mm(nc, fps[:], lgb[0:1, dc * 128:(dc + 1) * 128],
                            rstd[:], True, True)
                        eps_ = pp1.tile([128, 512], F32, tag="bc1")
                        _mm(nc, eps_[:], lgb[:, dc * 128:(dc + 1) * 128],
                            cr[:], True, True)
                        u = sc.tile([128, 512], F32, tag="u")
                        nc.vector.tensor_mul(
                            u[:], xt[:, dc * 512:(dc + 1) * 512].bitcast(F32),
                            fps[:])
                        nc.vector.tensor_add(yt[:, dc * 512:(dc + 1) * 512],
                                             u[:], eps_[:])
                    ht = ap.tile([128, 16 * 512], F32R, tag="ht")
                    for mo in range(16):
                        hps = pp.tile([128, 512], F32, tag="mm0")
                        for dc in range(4):
                            _mm(nc, hps[:],
                                f1t[:, dc * DFF + mo * 128: dc * DFF + (mo + 1) * 128],
                                yt[:, dc * 512:(dc + 1) * 512], dc == 0, dc == 3)
                        nc.scalar.activation(ht[:, mo * 512:(mo + 1) * 512],
                                             hps[:], RELU,
                                             bias=b1[:, mo:mo + 1])
                    ou = sc.tile([128, 2048], F32, tag="ou")
                    for mo in range(4):
                        ops2 = pp.tile([128, 512], F32, tag="mm1")
                        for dc in range(16):
                            _mm(nc, ops2[:],
                                f2t[:, dc * D + mo * 128: dc * D + (mo + 1) * 128],
                                ht[:, dc * 512:(dc + 1) * 512], dc == 0, dc == 15)
                        nc.vector.scalar_tensor_tensor(
                            ou[:, mo * 512:(mo + 1) * 512], ops2[:],
                            b2[:, mo:mo + 1],
                            xt[:, mo * 512:(mo + 1) * 512].bitcast(F32),
                            ADD, ADD)
                    nc.sync.dma_start(
                        out.rearrange("(mo p) f -> p mo f", p=128)
                        [:, :, s * 512:(s + 1) * 512], ou[:])
    nc.compile()
    return nc


_CACHED = None
LAST_RESULT = None


def _get_nc():
    global _CACHED
    if _CACHED is None:
        _CACHED = build()
    return _CACHED


def kernel(**inputs):
    x = np.asarray(inputs["x"], np.float32)          # [T, N, D]
    base = {}
    for p in ("s", "t"):
        for m in ("q", "k", "v", "o"):
            wm = np.asarray(inputs[f"{p}{m}_w"], np.float32)
            bm = np.asarray(inputs[f"{p}{m}_b"], np.float32)
            if m == "q":                              # fold 1/sqrt(DK)
                wm, bm = wm / np.sqrt(DK), bm / np.sqrt(DK)
            base[p + m + "w"] = np.ascontiguousarray(wm)
            if m == "v":
                base[p + "vbb"] = np.ascontiguousarray(
                    np.broadcast_to(bm, (128, D)))
            else:
                base[p + m + "b"] = np.ascontiguousarray(
                    bm.reshape(4, 128).T)
    base["f1w"] = np.ascontiguousarray(np.asarray(inputs["f1_w"], np.float32))
    base["f1b"] = np.ascontiguousarray(
        np.asarray(inputs["f1_b"], np.float32).reshape(16, 128).T)
    base["f2w"] = np.ascontiguousarray(np.asarray(inputs["f2_w"], np.float32))
    base["f2b"] = np.ascontiguousarray(
        np.asarray(inputs["f2_b"], np.float32).reshape(4, 128).T)
    base["lngb"] = np.ascontiguousarray(np.stack([
        np.asarray(inputs["ln3_g"], np.float32),
        np.asarray(inputs["ln3_b"], np.float32)]))
    base["ones512"] = np.ones((1, 512), np.float32)
    base["ones128"] = np.ones((128, 1), np.float32)

    in_maps = []
    for i in range(NC):
        m = dict(base)
        m["xs"] = np.ascontiguousarray(
            x[i * TS:(i + 1) * TS].reshape(TOK, D).T)
        in_maps.append(m)

    ncm = _get_nc()
    res = bass_utils.run_bass_kernel_spmd(
        ncm, in_maps, core_ids=list(range(NC)), trace=False)
    global LAST_RESULT
    LAST_RESULT = res
    o = np.stack([r["out"] for r in res.results])     # [8, D, TOK] f=(nl*T+t)
    return np.ascontiguousarray(
        o.reshape(NC, D, NS, T).transpose(3, 0, 2, 1).reshape(T, N, D))


# revision 17
# speedup vs baseline: 25.5521x; 25.5521x over previous
"""DecoderLayer (spatial attn + causal temporal attn + FFN) on 8 TRN2 NeuronCores.

Sharding: data-parallel. Spatial attention shards T (16 t-slices/core);
an on-device AllToAll reshards to N (32 cells/core) for causal temporal
attention + FFN. Activations are kept feature-major ([d, token]) on chip so
every matmul consumes natural layouts; the host pre/post-transposes x/out.

Matmul dtypes: fp32r (full-rate fp32 at free-dim>=256) for projections/FFN
and spatial scores; bf16 for softmax weights, V, and temporal scores.
Attention processes head PAIRS so softmax/normalize elementwise ops run at
the full 128-partition width.
"""
import sys

sys.path.insert(0, "/opt/trn_rl_repo")
import numpy as np
import concourse.bass as bass
import concourse.mybir as mybir
import concourse.tile as tile
from concourse import bacc
from concourse import bass_utils
from concourse.masks import make_upper_triangular

F32 = mybir.dt.float32
F32R = mybir.dt.float32r
BF16 = mybir.dt.bfloat16
EXP = mybir.ActivationFunctionType.Exp
SQUARE = mybir.ActivationFunctionType.Square
SQRT = mybir.ActivationFunctionType.Sqrt
IDENT = mybir.ActivationFunctionType.Identity
RELU = mybir.ActivationFunctionType.Relu
ADD = mybir.AluOpType.add
MULT = mybir.AluOpType.mult

T, N, D, H, DK, DFF = 128, 256, 512, 8, 64, 2048
NC = 8
TS = T // NC          # 16 t-slices per core (spatial)
NS = N // NC          # 32 cells per core (temporal)
TOK = TS * N          # 4096 tokens per core
NB = 8                # temporal batch: cells per batch
NBATCH = NS // NB     # 4 temporal batches
BTOK = T * NB         # 1024 tokens per temporal batch


def _mm(nc, out, lhsT, rhs, start, stop):
    nc.tensor.matmul(out, lhsT, rhs, start=start, stop=stop)


def _load_w(nc, sb, dram_ap, kdim, ndim, tag, dt=F32R):
    """Weight [kdim, ndim] -> SBUF [128, (kdim//128)*ndim], chunk dc at
    cols [dc*ndim, (dc+1)*ndim). Single multi-dim DMA."""
    kc = kdim // 128
    t = sb.tile([128, kc * ndim], dt, tag=tag, name=tag)
    nc.sync.dma_start(
        t[:], dram_ap.rearrange("(dc p) n -> p dc n", p=128).bitcast(dt))
    return t


def _attn_weights(nc, wp, w, p):
    wq = _load_w(nc, wp, w[p + "qw"], D, D, "wq")
    wk = _load_w(nc, wp, w[p + "kw"], D, D, "wk")
    wv = _load_w(nc, wp, w[p + "vw"], D, D, "wv")
    wo = _load_w(nc, wp, w[p + "ow"], D, D, "wo")
    bias = {}
    for m in ("q", "k", "o"):
        bt = wp.tile([128, 4], F32, tag="b" + m, name="b" + m)
        nc.sync.dma_start(bt[:], w[p + m + "b"])
        bias[m] = bt
    bvb = wp.tile([128, D], F32, tag="bvb", name="bvb")
    nc.sync.dma_start(bvb[:], w[p + "vbb"])
    return wq, wk, wv, wo, bias, bvb


def build():
    nc = bacc.Bacc("TRN2", target_bir_lowering=False, debug=False,
                   num_devices=NC)

    def inp(name, shape):
        return nc.dram_tensor(name, shape, F32, kind="ExternalInput").ap()

    xs = inp("xs", [D, TOK])                       # spatial shard, feature-major
    w = {}
    for p in ("s", "t"):
        for m in ("q", "k", "v", "o"):
            w[p + m + "w"] = inp(p + m + "w", [D, D])
        for m in ("q", "k", "o"):
            w[p + m + "b"] = inp(p + m + "b", [128, 4])
        w[p + "vbb"] = inp(p + "vbb", [128, D])
    f1w = inp("f1w", [D, DFF])
    f1b = inp("f1b", [128, DFF // 128])
    f2w = inp("f2w", [DFF, D])
    f2b = inp("f2b", [128, 4])
    lngb = inp("lngb", [2, D])                     # row0 = ln3_g, row1 = ln3_b
    ones512 = inp("ones512", [1, 512])
    ones128 = inp("ones128", [128, 1])
    out = nc.dram_tensor("out", [D, TOK], F32, kind="ExternalOutput").ap()

    with tile.TileContext(nc) as tc, \
         nc.allow_low_precision(reason="bf16 softmax weights and values"):
        with tc.tile_pool(name="const", bufs=1) as cb, \
             tc.tile_pool(name="dram", bufs=1, space="DRAM") as dram:
            ones_bf = cb.tile([128, 1], BF16, tag="obf")
            nc.vector.memset(ones_bf[:], 1.0)
            ones_r1 = cb.tile([1, 512], F32R, tag="o1r")
            nc.sync.dma_start(ones_r1[:], ones512.bitcast(F32R))
            ones_r128 = cb.tile([128, 1], F32R, tag="o128r")
            nc.sync.dma_start(ones_r128[:], ones128.bitcast(F32R))
            mask = cb.tile([128, 128], BF16, tag="mask")
            make_upper_triangular(nc, mask[:], val=1.0, diag=True)
            mask4 = cb.tile([128, 512], BF16, tag="mask4")
            for i in range(4):
                nc.vector.tensor_copy(mask4[:, i * 128:(i + 1) * 128], mask[:])
            lgb = cb.tile([2, D], F32R, tag="lgb")
            nc.sync.dma_start(lgb[:], lngb.bitcast(F32R))

            # a2a buffers: [dst, tloc(4), dc, p, n(32)] per quarter
            q_shape = [NC, TS // 4, 4, 128, 32]
            sends = [dram.tile(q_shape, F32, name=f"send{i}",
                               tag=f"send{i}") for i in range(4)]
            recvs = [dram.tile(q_shape, F32, name=f"recv{i}",
                               tag=f"recv{i}") for i in range(4)]

            # ---------------- Phase A: spatial attention over n (shard T) ---
            with tc.tile_pool(name="wA", bufs=1) as wp, \
                 tc.tile_pool(name="actA", bufs=2) as ap, \
                 tc.tile_pool(name="smA", bufs=4) as sc, \
                 tc.tile_pool(name="psA", bufs=2, space="PSUM") as pp:
                wq, wk, wv, wo, bias, bvb = _attn_weights(nc, wp, w, "s")

                for tp in range(TS // 2):   # 8 pairs of t-slices, 512 tok each
                    xt = ap.tile([128, 2048], F32R, tag="xt")
                    nc.sync.dma_start(
                        xt[:], xs.rearrange("(dc p) f -> p dc f", p=128)
                        [:, :, tp * 512:(tp + 1) * 512].bitcast(F32R))
                    qt = ap.tile([128, 2048], F32R, tag="qt")
                    kt = ap.tile([128, 2048], F32R, tag="kt")
                    for mo in range(4):
                        qps = pp.tile([128, 512], F32, tag="pa")
                        kps = pp.tile([128, 512], F32, tag="pb")
                        for dc in range(4):
                            _mm(nc, qps[:],
                                wq[:, dc * D + mo * 128: dc * D + (mo + 1) * 128],
                                xt[:, dc * 512:(dc + 1) * 512], dc == 0, dc == 3)
                            _mm(nc, kps[:],
                                wk[:, dc * D + mo * 128: dc * D + (mo + 1) * 128],
                                xt[:, dc * 512:(dc + 1) * 512], dc == 0, dc == 3)
                        nc.scalar.activation(qt[:, mo * 512:(mo + 1) * 512],
                                             qps[:], IDENT,
                                             bias=bias["q"][:, mo:mo + 1])
                        nc.vector.tensor_scalar_add(
                            kt[:, mo * 512:(mo + 1) * 512], kps[:],
                            bias["k"][:, mo:mo + 1])
                    # V token-major, bf16: chunk c (128 tokens) at cols c*512..
                    vt = ap.tile([128, 2048], BF16, tag="vt")
                    for c in range(4):
                        vps = pp.tile([128, 512], F32, tag="pa")
                        for dc in range(4):
                            _mm(nc, vps[:],
                                xt[:, dc * 512 + c * 128: dc * 512 + (c + 1) * 128],
                                wv[:, dc * D:(dc + 1) * D], dc == 0, dc == 3)
                        nc.vector.tensor_add(vt[:, c * 512:(c + 1) * 512],
                                             vps[:], bvb[:])
                    ot = ap.tile([128, 2048], F32R, tag="ot")
                    for hp in range(4):           # head pair chunk
                        for hh in range(2):
                            h = 2 * hp + hh
                            po = 64 * hh
                            es_d = {}
                            dps = pp.tile([1, 512], F32, tag="pd")
                            for tl in range(2):
                                base = hp * 512 + tl * 256
                                sps = pp.tile([128, 512], F32, tag="pb")
                                for kc in range(2):
                                    _mm(nc, sps[:, kc * 256:(kc + 1) * 256],
                                        kt[po:po + 64,
                                           base + kc * 128: base + (kc + 1) * 128],
                                        qt[po:po + 64, base:base + 256],
                                        True, True)
                                es = sc.tile([128, 512], BF16, tag="es")
                                nc.scalar.activation(es[:], sps[:], EXP)
                                for kc in range(2):
                                    _mm(nc, dps[:, tl * 256:(tl + 1) * 256],
                                        ones_bf[:],
                                        es[:, kc * 256:(kc + 1) * 256],
                                        kc == 0, kc == 1)
                                es_d[tl] = es
                            recf = sc.tile([1, 512], F32, tag="recf")
                            nc.vector.reciprocal_approx_fast(recf[:], dps[:])
                            rec = sc.tile([1, 512], F32R, tag="rec")
                            nc.scalar.copy(rec[:], recf[:])
                            bps = pp.tile([64, 512], F32, tag="pe")
                            _mm(nc, bps[:], ones_r1[:, :64], rec[:], True, True)
                            ops = pp.tile([64, 512], F32, tag="pa")
                            for tl in range(2):
                                for kc in range(2):
                                    _mm(nc, ops[:, tl * 256:(tl + 1) * 256],
                                        vt[:, (2 * tl + kc) * 512 + h * 64:
                                           (2 * tl + kc) * 512 + (h + 1) * 64],
                                        es_d[tl][:, kc * 256:(kc + 1) * 256],
                                        kc == 0, kc == 1)
                            osl = ot[po:po + 64, hp * 512:(hp + 1) * 512]
                            nc.scalar.copy(osl, ops[:])
                            nc.vector.tensor_mul(osl, osl, bps[:])
                    r1 = ap.tile([128, 2048], F32, tag="r1")
                    for mo in range(4):
                        rps = pp.tile([128, 512], F32, tag="pb")
                        for dc in range(4):
                            _mm(nc, rps[:],
                                wo[:, dc * D + mo * 128: dc * D + (mo + 1) * 128],
                                ot[:, dc * 512:(dc + 1) * 512], dc == 0, dc == 3)
                        nc.vector.scalar_tensor_tensor(
                            r1[:, mo * 512:(mo + 1) * 512], rps[:],
                            bias["o"][:, mo:mo + 1],
                            xt[:, mo * 512:(mo + 1) * 512].bitcast(F32),
                            ADD, ADD)
                    # scatter to a2a send buffers: 1 DMA per t-slice
                    r1v = r1.rearrange("p (dc tl j n) -> p tl dc j n",
                                       dc=4, tl=2, j=8)
                    for tl in range(2):
                        t = 2 * tp + tl
                        sh, tloc = t // 4, t % 4
                        for j in range(NC):
                            nc.sync.dma_start(
                                sends[sh][j, tloc].transpose([1, 0, 2]),
                                r1v[:, tl, :, j])
                    if tp % 2 == 1:
                        q = tp // 2
                        nc.gpsimd.collective_compute(
                            "AllToAll", mybir.AluOpType.bypass,
                            replica_groups=[list(range(NC))],
                            ins=[sends[q][:].opt()], outs=[recvs[q][:].opt()])

            # ---------------- Phase B: temporal attention over t (shard N) --
            with tc.tile_pool(name="wB", bufs=1) as wp, \
                 tc.tile_pool(name="actBx", bufs=2) as apx, \
                 tc.tile_pool(name="actB", bufs=1) as ap, \
                 tc.tile_pool(name="smB", bufs=3) as sc, \
                 tc.tile_pool(name="smB2", bufs=2) as sc2, \
                 tc.tile_pool(name="smB3", bufs=1) as sc3, \
                 tc.tile_pool(name="psB", bufs=2, space="PSUM") as pp:
                wq, wk, wv, wo, bias, bvb = _attn_weights(nc, wp, w, "t")
                # FFN weights in bf16 (cast on DMA)
                f1t = wp.tile([128, 4 * DFF], BF16, tag="f1", name="f1")
                nc.gpsimd.dma_start(
                    f1t[:], f1w.rearrange("(dc p) n -> p dc n", p=128))
                f2t = wp.tile([128, 16 * D], BF16, tag="f2", name="f2")
                nc.gpsimd.dma_start(
                    f2t[:], f2w.rearrange("(dc p) n -> p dc n", p=128))
                b1 = wp.tile([128, 16], F32, tag="b1", name="b1")
                nc.sync.dma_start(b1[:], f1b)
                b2 = wp.tile([128, 4], F32, tag="b2", name="b2")
                nc.sync.dma_start(b2[:], f2b)

                for b in range(NBATCH):   # 8 cells per batch, 1024 tokens
                    xt = apx.tile([128, 4 * BTOK], F32R, tag="xt")
                    xtv = xt.rearrange("p (dc s2 q4 tl nl) -> p dc s2 q4 tl nl",
                                       dc=4, s2=8, q4=4, nl=NB)
                    for src in range(NC):
                        for qr in range(4):
                            for dc in range(4):
                                nc.sync.dma_start(
                                    xtv[:, dc, src, qr],
                                    recvs[qr][src, :, dc, :,
                                              b * NB:(b + 1) * NB]
                                    .transpose([1, 0, 2]).bitcast(F32R))
                    qt = ap.tile([128, 4 * BTOK], BF16, tag="qt")
                    kt = ap.tile([128, 4 * BTOK], BF16, tag="kt")
                    for mo in range(4):
                        for sl in range(2):
                            qps = pp.tile([128, 512], F32, tag="pa")
                            kps = pp.tile([128, 512], F32, tag="pb")
                            for dc in range(4):
                                _mm(nc, qps[:],
                                    wq[:, dc * D + mo * 128: dc * D + (mo + 1) * 128],
                                    xt[:, dc * BTOK + sl * 512:
                                       dc * BTOK + (sl + 1) * 512], dc == 0, dc == 3)
                                _mm(nc, kps[:],
                                    wk[:, dc * D + mo * 128: dc * D + (mo + 1) * 128],
                                    xt[:, dc * BTOK + sl * 512:
                                       dc * BTOK + (sl + 1) * 512], dc == 0, dc == 3)
                            nc.scalar.activation(
                                qt[:, mo * BTOK + sl * 512: mo * BTOK + (sl + 1) * 512],
                                qps[:], IDENT, bias=bias["q"][:, mo:mo + 1])
                            nc.vector.tensor_scalar_add(
                                kt[:, mo * BTOK + sl * 512: mo * BTOK + (sl + 1) * 512],
                                kps[:], bias["k"][:, mo:mo + 1])
                    vt = ap.tile([128, NB * 512], BF16, tag="vt")
                    for nl in range(NB):
                        vps = pp.tile([128, 512], F32, tag="pa")
                        for dc in range(4):
                            lhsT = xt[:, dc * BTOK:(dc + 1) * BTOK].rearrange(
                                "p (t n) -> p t n", n=NB)[:, :, nl]
                            _mm(nc, vps[:], lhsT, wv[:, dc * D:(dc + 1) * D],
                                dc == 0, dc == 3)
                        nc.vector.tensor_add(vt[:, nl * 512:(nl + 1) * 512],
                                             vps[:], bvb[:])
                    ot = ap.tile([128, 4 * BTOK], F32R, tag="ot")
                    for quad in range(NB // 4):
                        n1 = 4 * quad
                        for h in range(H):
                            po, dch = (h % 2) * 64, h // 2
                            qh = qt[:, dch * BTOK:(dch + 1) * BTOK].rearrange(
                                "p (t n) -> p t n", n=NB)
                            kh = kt[:, dch * BTOK:(dch + 1) * BTOK].rearrange(
                                "p (t n) -> p t n", n=NB)
                            # sps blocks: 4 cells of head h
                            sps = pp.tile([128, 512], F32, tag="pb")
                            for j in range(4):
                                _mm(nc, sps[:, j * 128:(j + 1) * 128],
                                    kh[po:po + 64, :, n1 + j],
                                    qh[po:po + 64, :, n1 + j], True, True)
                            es = sc.tile([128, 512], BF16, tag="es")
                            nc.scalar.activation(es[:], sps[:], EXP)
                            esm = sc.tile([128, 512], BF16, tag="esm")
                            nc.vector.tensor_mul(esm[:], es[:], mask4[:])
                            dps = pp.tile([1, 512], F32, tag="pd")
                            _mm(nc, dps[:], ones_bf[:], esm[:], True, True)
                            recf = sc.tile([1, 512], F32, tag="recf")
                            nc.vector.reciprocal_approx_fast(recf[:], dps[:])
                            rec = sc.tile([1, 512], F32R, tag="rec")
                            nc.scalar.copy(rec[:], recf[:])
                            bps = pp.tile([64, 512], F32, tag="pe")
                            _mm(nc, bps[:], ones_r1[:, :64], rec[:], True, True)
                            ops = pp.tile([64, 512], F32, tag="pa")
                            for j in range(4):
                                _mm(nc, ops[:, j * 128:(j + 1) * 128],
                                    vt[:, (n1 + j) * 512 + h * 64:
                                       (n1 + j) * 512 + (h + 1) * 64],
                                    esm[:, j * 128:(j + 1) * 128], True, True)
                            osl = ot[:, dch * BTOK:(dch + 1) * BTOK].rearrange(
                                "p (t n) -> p n t", n=NB)[po:po + 64,
                                                          n1:n1 + 4, :]
                            nc.scalar.copy(osl, ops[:])
                            nc.vector.tensor_mul(osl, osl, bps[:])
                    r2 = ap.tile([128, 4 * BTOK], F32R, tag="r2")
                    for mo in range(4):
                        for sl in range(2):
                            rps = pp.tile([128, 512], F32, tag="pb")
                            for dc in range(4):
                                _mm(nc, rps[:],
                                    wo[:, dc * D + mo * 128: dc * D + (mo + 1) * 128],
                                    ot[:, dc * BTOK + sl * 512:
                                       dc * BTOK + (sl + 1) * 512], dc == 0, dc == 3)
                            nc.vector.scalar_tensor_tensor(
                                r2[:, mo * BTOK + sl * 512: mo * BTOK + (sl + 1) * 512],
                                rps[:], bias["o"][:, mo:mo + 1],
                                xt[:, mo * BTOK + sl * 512:
                                   mo * BTOK + (sl + 1) * 512].bitcast(F32),
                                ADD, ADD)
                    # ---- fused LN + FFN on this batch (2 slabs of 512) --
                    for sl in range(2):
                        xsl = [r2[:, dc * BTOK + sl * 512:
                                  dc * BTOK + (sl + 1) * 512] for dc in range(4)]
                        sps = pp.tile([1, 512], F32, tag="pd")
                        sqps = pp.tile([1, 512], F32, tag="pd")
                        for dc in range(4):
                            sq = sc2.tile([128, 512], F32R, tag="sq")
                            nc.scalar.activation(sq[:], xsl[dc].bitcast(F32),
                                                 SQUARE)
                            _mm(nc, sps[:], ones_r128[:], xsl[dc],
                                dc == 0, dc == 3)
                            _mm(nc, sqps[:], ones_r128[:], sq[:],
                                dc == 0, dc == 3)
                        nmean = sc3.tile([1, 512], F32, tag="nm")
                        nc.vector.tensor_scalar_mul(nmean[:], sps[:], -1.0 / D)
                        tvar = sc3.tile([1, 512], F32, tag="tv")
                        nc.vector.tensor_scalar_mul(tvar[:], sqps[:], 1.0 / D)
                        m2 = sc3.tile([1, 512], F32, tag="m2")
                        nc.vector.tensor_mul(m2[:], nmean[:], nmean[:])
                        nc.vector.tensor_sub(tvar[:], tvar[:], m2[:])
                        nc.vector.tensor_scalar_add(tvar[:], tvar[:], 1e-5)
                        sd = sc3.tile([1, 512], F32, tag="sd")
                        nc.scalar.activation(sd[:], tvar[:], SQRT)
                        rstdf = sc3.tile([1, 512], F32, tag="rsf")
                        nc.vector.reciprocal_approx_fast(rstdf[:], sd[:])
                        rstd = sc3.tile([1, 512], F32R, tag="rs")
                        nc.scalar.copy(rstd[:], rstdf[:])
                        cr = sc3.tile([2, 512], F32R, tag="cr")
                        nc.vector.tensor_mul(cr[0:1, :], nmean[:], rstdf[:])
                        nc.sync.dma_start(cr[1:2, :], ones512.bitcast(F32R))
                        yt = ap.tile([128, 2048], BF16, tag="qt")
                        for dc in range(4):
                            fps = pp.tile([128, 512], F32, tag="pe")
                            _mm(nc, fps[:], lgb[0:1, dc * 128:(dc + 1) * 128],
                                rstd[:], True, True)
                            eps_ = pp.tile([128, 512], F32, tag="pe")
                            _mm(nc, eps_[:], lgb[:, dc * 128:(dc + 1) * 128],
                                cr[:], True, True)
                            u = sc2.tile([128, 512], F32, tag="u")
                            nc.vector.tensor_mul(u[:], xsl[dc].bitcast(F32),
                                                 fps[:])
                            nc.vector.tensor_add(
                                yt[:, dc * 512:(dc + 1) * 512], u[:], eps_[:])
                        ht = ap.tile([128, 16 * 512], BF16, tag="ot")
                        for mo in range(16):
                            hps = pp.tile([128, 512], F32, tag="pa")
                            for dc in range(4):
                                _mm(nc, hps[:],
                                    f1t[:, dc * DFF + mo * 128:
                                        dc * DFF + (mo + 1) * 128],
                                    yt[:, dc * 512:(dc + 1) * 512],
                                    dc == 0, dc == 3)
                            nc.scalar.activation(
                                ht[:, mo * 512:(mo + 1) * 512], hps[:], RELU,
                                bias=b1[:, mo:mo + 1])
                        ou = ap.tile([128, 2048], F32, tag="kt")
                        for mo in range(4):
                            ops2 = pp.tile([128, 512], F32, tag="pb")
                            for dc in range(16):
                                _mm(nc, ops2[:],
                                    f2t[:, dc * D + mo * 128:
                                        dc * D + (mo + 1) * 128],
                                    ht[:, dc * 512:(dc + 1) * 512],
                                    dc == 0, dc == 15)
                            nc.vector.scalar_tensor_tensor(
                                ou[:, mo * 512:(mo + 1) * 512], ops2[:],
                                b2[:, mo:mo + 1], xsl[mo].bitcast(F32),
                                ADD, ADD)
                        # out f = b*1024 + sl*512 + (t*8+nl)  (contiguous)
                        nc.sync.dma_start(
                            out.rearrange("(mo p) f -> p mo f", p=128)
                            [:, :, b * BTOK + sl * 512:
                             b * BTOK + (sl + 1) * 512], ou[:])
    nc.compile()
    return nc


_CACHED = None
LAST_RESULT = None


def _get_nc():
    global _CACHED
    if _CACHED is None:
        _CACHED = build()
    return _CACHED


def kernel(**inputs):
    x = np.asarray(inputs["x"], np.float32)          # [T, N, D]
    base = {}
    for p in ("s", "t"):
        for m in ("q", "k", "v", "o"):
            wm = np.asarray(inputs[f"{p}{m}_w"], np.float32)
            bm = np.asarray(inputs[f"{p}{m}_b"], np.float32)
            if m == "q":                              # fold 1/sqrt(DK)
                wm, bm = wm / np.sqrt(DK), bm / np.sqrt(DK)
            base[p + m + "w"] = np.ascontiguousarray(wm)
            if m == "v":
                base[p + "vbb"] = np.ascontiguousarray(
                    np.broadcast_to(bm, (128, D)))
            else:
                base[p + m + "b"] = np.ascontiguousarray(
                    bm.reshape(4, 128).T)
    base["f1w"] = np.ascontiguousarray(np.asarray(inputs["f1_w"], np.float32))
    base["f1b"] = np.ascontiguousarray(
        np.asarray(inputs["f1_b"], np.float32).reshape(16, 128).T)
    base["f2w"] = np.ascontiguousarray(np.asarray(inputs["f2_w"], np.float32))
    base["f2b"] = np.ascontiguousarray(
        np.asarray(inputs["f2_b"], np.float32).reshape(4, 128).T)
    base["lngb"] = np.ascontiguousarray(np.stack([
        np.asarray(inputs["ln3_g"], np.float32),
        np.asarray(inputs["ln3_b"], np.float32)]))
    base["ones512"] = np.ones((1, 512), np.float32)
    base["ones128"] = np.ones((128, 1), np.float32)

    in_maps = []
    for i in range(NC):
        m = dict(base)
        m["xs"] = np.ascontiguousarray(
            x[i * TS:(i + 1) * TS].reshape(TOK, D).T)
        in_maps.append(m)

    ncm = _get_nc()
    res = bass_utils.run_bass_kernel_spmd(
        ncm, in_maps, core_ids=list(range(NC)), trace=False)
    global LAST_RESULT
    LAST_RESULT = res
    o = np.stack([r["out"] for r in res.results])  # [8,D,TOK] f=b*1024+t*8+nl
    return np.ascontiguousarray(
        o.reshape(NC, D, NBATCH, T, NB).transpose(3, 0, 2, 4, 1)
        .reshape(T, N, D))
